# revision 13
# baseline (speedup 1.0000x reference)
"""GNN message-passing kernel for Trainium2 (8 NeuronCores).

Reference computation:
    out[b,i,f] = X[b,0,i,i,f] + sum_{k=1..3} sum_j A[b,i,j] * X[b,k,i,j,f]

Sharding: 8 cores = (batch b in 0..3) x (i-half h in 0..1); each core owns
a (b, 128-row i-slab) of the output. Hop 0 only contributes its diagonal,
so only X[b,1:4] plus the hop-0 diagonal rows are sent to the device.

Precision: the harness gate is rel_err < 2e-2; fp32 is ~1.9e-7, so X and A
are downcast to fp16 on the host (~6e-4 final rel err). This halves HBM
traffic (12.6 MB/core) and unlocks the DVE's 2x packed mode.

Layout: X slabs are pre-transposed on the host to f-major [k, i, f, j] so
the j-reduction is innermost-contiguous.

DMA: the Tile framework only allows ~9 outstanding DMA instructions
(completion-semaphore pool), and each queue sustains only ~270 GB/s, so
the kernel fuses each f-chunk's three hop tiles into ONE dma_start (3D
access pattern over k) -> 7 DMA instructions for all of X, all in flight
at once (no mid-stream dispatch refills), alternating between the SP and
Activation hardware DGE queues (the 16 shared DMA engines burst ~25 GB/s
each; two continuously-fed queues approach the ~400 GB/s pool limit).

Compute per f-chunk:
  - PE sums the 3 hops via identity-stationary fp16 matmuls (single-pass,
    full rate) accumulating into PSUM fp32, 512-col slices.
  - ACT copies PSUM -> SBUF fp16 per 2048-col piece.
  - DVE multiplies by A broadcast over f (2x mode: broadcast rides the
    y-dim) in place, then reduces over j with a 3-level contiguous add-tree
    (2x mode; TENSOR_REDUCE has no packed mode so it only handles the final
    32->1 step) and adds the hop-0 diagonal into the fp32 accumulator.
"""

import sys

if "/opt/trn_rl_repo" not in sys.path:
    sys.path.insert(0, "/opt/trn_rl_repo")

import numpy as np

import concourse.bacc as bacc
import concourse.bass as bass
import concourse.mybir as mybir
from concourse.bass_utils import run_bass_kernel_spmd
from concourse.tile import TileContext

BATCH, KP1, N, F = 4, 4, 256, 64
NH = N // 2          # 128 rows of output per core (partition dim)
# f-chunk sizes (sum = F). Small first chunk -> fast pipeline fill; small
# last chunks -> short drain tail.
CFS = [2, 4, 8, 8, 8, 8, 8, 8, 6, 2, 2]
PIECE = 2048         # PSUM tile free width (8 KB fp32 = 4 banks)
FP32 = mybir.dt.float32
FP16 = mybir.dt.float16

_CACHE = {}


def _build_nc():
    if "nc" in _CACHE:
        return _CACHE["nc"]
    nc = bacc.Bacc("TRN2", target_bir_lowering=False, debug=False, num_devices=8)
    xk = nc.dram_tensor("xk", [3, NH, F, N], FP16, kind="ExternalInput").ap()
    a = nc.dram_tensor("a", [NH, N], FP16, kind="ExternalInput").ap()
    d = nc.dram_tensor("d", [NH, F], FP32, kind="ExternalInput").ap()
    eye = nc.dram_tensor("eye", [128, 128], FP16, kind="ExternalInput").ap()
    out = nc.dram_tensor("out", [NH, F], FP32, kind="ExternalOutput").ap()

    with TileContext(nc) as tc:
        with (
            tc.tile_pool(name="const", bufs=1) as cpool,
            tc.tile_pool(name="xs", bufs=1) as xpool,
            tc.tile_pool(name="sm", bufs=4) as spool,
            tc.tile_pool(name="tr", bufs=3) as tpool,
            tc.tile_pool(name="ac", bufs=1) as acpool,
            tc.tile_pool(name="ps", bufs=2, space="PSUM") as pspool,
        ):
            eye_sb = cpool.tile([128, 128], FP16)
            nc.sync.dma_start(out=eye_sb[:, :], in_=eye[:, :])
            a_sb = cpool.tile([128, N], FP16)
            d_sb = cpool.tile([128, F], FP32)

            acc = acpool.tile([128, F], FP32)
            a_step = a_sb.ap[0][0]

            # One fused DMA per chunk: 3D AP pulls all three hop slabs.
            # All X dispatches go out up front on the SP queue in chunk
            # order (a deeply-backlogged single queue sustains ~410 GB/s).
            # a/d are interleaved after the first X chunks: they are not
            # needed until the first DVE mul / final accumulate.
            xts = []
            f0 = 0
            for c, CF in enumerate(CFS):
                CW = CF * N
                xt = xpool.tile([128, 3 * CW], FP16, name=f"x{c}", tag=f"x{c}")
                xt_step = xt.ap[0][0]
                dst = bass.AP(xt.tensor, 0, [[xt_step, 128], [CW, 3], [1, CW]])
                src = bass.AP(
                    xk.tensor,
                    f0 * N,
                    [[F * N, 128], [NH * F * N, 3], [1, CW]],
                )
                nc.sync.dma_start(out=dst, in_=src)
                xts.append(xt)
                f0 += CF
                if c == 1:
                    nc.sync.dma_start(out=a_sb[:, :], in_=a[:, :])
                    nc.sync.dma_start(out=d_sb[:, :], in_=d[:, :])

            # PE warm-up while the first X DMAs are in flight: trips the
            # HAM activity window and bridges the fill gap so real matmuls
            # run at full clock from the start.
            warm = pspool.tile([128, 512], FP32, name="ps", tag="ps")
            for _ in range(18):
                nc.tensor.matmul(
                    warm[:, 0:128], eye_sb[:, :], eye_sb[:, :],
                    start=True, stop=True,
                )

            f0 = 0
            for c, CF in enumerate(CFS):
                CW = CF * N
                xt = xts[c]
                # hop sum on PE -> PSUM fp32, then ACT -> one fp16 tile
                sm = spool.tile([128, CW], FP16, name="sm", tag="sm")
                sm_step = sm.ap[0][0]
                for p0 in range(0, CW, PIECE):
                    pw = min(PIECE, CW - p0)
                    ps = pspool.tile([128, pw], FP32, name="ps", tag="ps")
                    for s in range(pw // 512):
                        sl = slice(s * 512, (s + 1) * 512)
                        for k in range(3):
                            o = k * CW + p0 + s * 512
                            nc.tensor.matmul(
                                ps[:, sl],
                                eye_sb[:, :],
                                xt[:, o : o + 512],
                                start=(k == 0),
                                stop=(k == 2),
                            )
                    nc.scalar.copy(sm[:, p0 : p0 + pw], ps[:, :])

                if c < 2:
                    # keep PE duty high through the small ramp chunks so
                    # HAM does not drop the clock back to the cold p-state
                    for _ in range(5):
                        nc.tensor.matmul(
                            warm[:, 0:128], eye_sb[:, :], eye_sb[:, :],
                            start=True, stop=True,
                        )

                # sm[i, f*N + j] *= A[i, j]  (in place, 2x mode)
                smi = bass.AP(sm.tensor, 0, [[sm_step, 128], [N, CF], [1, N]])
                ab = bass.AP(a_sb.tensor, 0, [[a_step, 128], [0, CF], [1, N]])
                nc.vector.tensor_mul(smi, smi, ab)

                # j-reduction: 3 tree levels (2x mode) + TENSOR_REDUCE 32->1
                tree = tpool.tile([128, 224 * CF], FP16, name="tree", tag="tree")
                t_step = tree.ap[0][0]
                src_t, src_step, src_off, run = sm.tensor, sm_step, 0, N
                dst_off = 0
                for _ in range(3):
                    half = run // 2
                    i0 = bass.AP(
                        src_t, src_off, [[src_step, 128], [run, CF], [1, half]]
                    )
                    i1 = bass.AP(
                        src_t, src_off + half,
                        [[src_step, 128], [run, CF], [1, half]],
                    )
                    o = bass.AP(
                        tree.tensor, dst_off,
                        [[t_step, 128], [half, CF], [1, half]],
                    )
                    nc.vector.tensor_add(o, i0, i1)
                    src_t, src_step, src_off = tree.tensor, t_step, dst_off
                    dst_off += half * CF
                    run = half

                rin = bass.AP(
                    src_t, src_off, [[src_step, 128], [run, CF], [1, run]]
                )
                nc.vector.reduce_sum(
                    acc[:, f0 : f0 + CF], rin, axis=mybir.AxisListType.X
                )
                f0 += CF

            # single hop-0 diagonal add, then write out
            nc.vector.tensor_add(acc[:, :], acc[:, :], d_sb[:, :])
            nc.sync.dma_start(out=out[:, :], in_=acc[:, :])

    nc.compile()
    _CACHE["nc"] = nc
    return nc


def _make_in_maps(A, X):
    idx = np.arange(NH)
    eye16 = np.eye(128, dtype=np.float16)
    X16 = X[:, 1:4].astype(np.float16)  # (batch, 3, N, N, F)
    in_maps = []
    for c in range(8):
        b, h = c // 2, c % 2
        lo = h * NH
        # [k, i, j, f] -> [k, i, f, j] so j is innermost on the device
        xkT = np.ascontiguousarray(
            X16[b, :, lo : lo + NH].transpose(0, 1, 3, 2)
        )
        av = A[b, lo : lo + NH, :].astype(np.float16)
        dv = np.ascontiguousarray(X[b, 0, lo + idx, lo + idx, :])
        in_maps.append({"xk": xkT, "a": av, "d": dv, "eye": eye16})
    return in_maps


def run(A, X, trace=False, **kw):
    nc = _build_nc()
    in_maps = _make_in_maps(A, X)
    res = run_bass_kernel_spmd(
        nc, in_maps, core_ids=list(range(8)), trace=trace, **kw
    )
    out = np.empty((BATCH, N, F), dtype=np.float32)
    for c in range(8):
        b, h = c // 2, c % 2
        out[b, h * NH : (h + 1) * NH] = res.results[c]["out"]
    return out, res


def kernel(A, X):
    A = np.asarray(A, dtype=np.float32)
    X = np.asarray(X, dtype=np.float32)
    out, _ = run(A, X, trace=False)
    return out


# revision 14
# speedup vs baseline: 1.1163x; 1.1163x over previous
"""GNN message-passing kernel for Trainium2 (8 NeuronCores).

Reference computation:
    out[b,i,f] = X[b,0,i,i,f] + sum_{k=1..3} sum_j A[b,i,j] * X[b,k,i,j,f]

Sharding: 8 cores = (batch b in 0..3) x (i-half h in 0..1); each core owns
a (b, 128-row i-slab) of the output. Hop 0 only contributes its diagonal,
so only X[b,1:4] plus the hop-0 diagonal rows are sent to the device.

Precision: the harness gate is rel_err < 2e-2; fp32 is ~1.9e-7, so X and A
are downcast to fp16 on the host (~6e-4 final rel err). This halves HBM
traffic (12.6 MB/core) and unlocks the DVE's 2x packed mode.

Layout: X slabs are pre-transposed on the host to f-major [k, i, f, j] so
the j-reduction is innermost-contiguous.

DMA: a single deeply-backlogged SP hardware queue sustains ~410 GB/s
(16 shared engines x ~26 GB/s burst). Each f-chunk's three hop slabs are
fused into ONE dma_start (3D access pattern over k) so the ~9-entry
in-flight DMA window covers the whole stream; all dispatches are issued
up front in chunk order.

Compute per f-chunk:
  - PE sums the 3 hops via identity-stationary fp16 matmuls (single-pass,
    full rate) accumulating into PSUM fp32, 512-col slices.
  - ACT copies PSUM -> SBUF fp16 per 2048-col piece.
  - DVE multiplies by A broadcast over f (2x mode: broadcast rides the
    y-dim) in place, then reduces over j with a 3-level contiguous add-tree
    (2x mode; TENSOR_REDUCE has no packed mode so it only handles the final
    32->1 step) and adds the hop-0 diagonal into the fp32 accumulator.
"""

import sys

if "/opt/trn_rl_repo" not in sys.path:
    sys.path.insert(0, "/opt/trn_rl_repo")

import numpy as np

import concourse.bacc as bacc
import concourse.bass as bass
import concourse.mybir as mybir
from concourse.bass_utils import run_bass_kernel_spmd
from concourse.tile import TileContext

BATCH, KP1, N, F = 4, 4, 256, 64
NH = N // 2          # 128 rows of output per core (partition dim)
# f-chunk sizes (sum = F). Small first chunks -> fast pipeline fill +
# HAM ramp; small last chunks -> short drain tail.
CFS = [2, 4, 8, 8, 8, 8, 8, 8, 6, 2, 2]
PIECE = 2048         # PSUM tile free width (8 KB fp32 = 4 banks)
FP32 = mybir.dt.float32
FP16 = mybir.dt.float16

_CACHE = {}


def _build_nc():
    if "nc" in _CACHE:
        return _CACHE["nc"]
    nc = bacc.Bacc("TRN2", target_bir_lowering=False, debug=False, num_devices=8)
    xk = nc.dram_tensor("xk", [3, NH, F, N], FP16, kind="ExternalInput").ap()
    a = nc.dram_tensor("a", [NH, N], FP16, kind="ExternalInput").ap()
    d = nc.dram_tensor("d", [NH, F], FP32, kind="ExternalInput").ap()
    eye = nc.dram_tensor("eye", [128, 128], FP16, kind="ExternalInput").ap()
    out = nc.dram_tensor("out", [NH, F], FP32, kind="ExternalOutput").ap()

    with TileContext(nc) as tc:
        with (
            tc.tile_pool(name="const", bufs=1) as cpool,
            tc.tile_pool(name="xs", bufs=1) as xpool,
            tc.tile_pool(name="sm", bufs=4) as spool,
            tc.tile_pool(name="tr", bufs=3) as tpool,
            tc.tile_pool(name="st", bufs=3) as stpool,
            tc.tile_pool(name="ac", bufs=1) as acpool,
            tc.tile_pool(name="ps", bufs=2, space="PSUM") as pspool,
        ):
            eye_sb = cpool.tile([128, 128], FP16)
            nc.sync.dma_start(out=eye_sb[:, :], in_=eye[:, :])
            a_sb = cpool.tile([128, N], FP16)
            d_sb = cpool.tile([128, F], FP32)

            acc = acpool.tile([128, F], FP32)
            a_step = a_sb.ap[0][0]

            # One fused DMA per chunk: 3D AP pulls all three hop slabs.
            # All X dispatches go out up front on the SP queue in chunk
            # order. a/d are interleaved after the first X chunks: they
            # are not needed until the first DVE mul / accumulate.
            xts = []
            f0 = 0
            for c, CF in enumerate(CFS):
                CW = CF * N
                xt = xpool.tile([128, 3 * CW], FP16, name=f"x{c}", tag=f"x{c}")
                xt_step = xt.ap[0][0]
                dst = bass.AP(xt.tensor, 0, [[xt_step, 128], [CW, 3], [1, CW]])
                src = bass.AP(
                    xk.tensor,
                    f0 * N,
                    [[F * N, 128], [NH * F * N, 3], [1, CW]],
                )
                nc.sync.dma_start(out=dst, in_=src)
                xts.append(xt)
                f0 += CF
                if c == 1:
                    nc.sync.dma_start(out=a_sb[:, :], in_=a[:, :])
                    nc.sync.dma_start(out=d_sb[:, :], in_=d[:, :])

            # PE warm-up while the first X DMAs are in flight: trips the
            # HAM activity window and bridges the fill gap so real matmuls
            # run at full clock from the start.
            warm = pspool.tile([128, 512], FP32, name="ps", tag="ps")
            for _ in range(18):
                nc.tensor.matmul(
                    warm[:, 0:128], eye_sb[:, :], eye_sb[:, :],
                    start=True, stop=True,
                )

            f0 = 0
            for c, CF in enumerate(CFS):
                CW = CF * N
                xt = xts[c]
                # hop sum on PE -> PSUM fp32, then ACT -> one fp16 tile
                sm = spool.tile([128, CW], FP16, name="sm", tag="sm")
                sm_step = sm.ap[0][0]
                for p0 in range(0, CW, PIECE):
                    pw = min(PIECE, CW - p0)
                    ps = pspool.tile([128, pw], FP32, name="ps", tag="ps")
                    for s in range(pw // 512):
                        sl = slice(s * 512, (s + 1) * 512)
                        for k in range(3):
                            o = k * CW + p0 + s * 512
                            nc.tensor.matmul(
                                ps[:, sl],
                                eye_sb[:, :],
                                xt[:, o : o + 512],
                                start=(k == 0),
                                stop=(k == 2),
                            )
                    nc.scalar.copy(sm[:, p0 : p0 + pw], ps[:, :])

                # sm[i, f*N + j] *= A[i, j]  (in place, 2x mode)
                smi = bass.AP(sm.tensor, 0, [[sm_step, 128], [N, CF], [1, N]])
                ab = bass.AP(a_sb.tensor, 0, [[a_step, 128], [0, CF], [1, N]])
                nc.vector.tensor_mul(smi, smi, ab)

                # j-reduction: 3 tree levels (2x mode) + TENSOR_REDUCE 32->1
                tree = tpool.tile([128, 224 * CF], FP16, name="tree", tag="tree")
                t_step = tree.ap[0][0]
                src_t, src_step, src_off, run = sm.tensor, sm_step, 0, N
                dst_off = 0
                for _ in range(3):
                    half = run // 2
                    i0 = bass.AP(
                        src_t, src_off, [[src_step, 128], [run, CF], [1, half]]
                    )
                    i1 = bass.AP(
                        src_t, src_off + half,
                        [[src_step, 128], [run, CF], [1, half]],
                    )
                    o = bass.AP(
                        tree.tensor, dst_off,
                        [[t_step, 128], [half, CF], [1, half]],
                    )
                    nc.vector.tensor_add(o, i0, i1)
                    src_t, src_step, src_off = tree.tensor, t_step, dst_off
                    dst_off += half * CF
                    run = half

                stage = stpool.tile([128, CF], FP32, name="stage", tag="stage")
                rin = bass.AP(
                    src_t, src_off, [[src_step, 128], [run, CF], [1, run]]
                )
                nc.vector.reduce_sum(stage[:, :], rin, axis=mybir.AxisListType.X)

                # acc[:, f0:f0+CF] = stage + hop-0 diagonal
                nc.vector.tensor_add(
                    acc[:, f0 : f0 + CF], stage[:, :], d_sb[:, f0 : f0 + CF]
                )
                f0 += CF

            nc.sync.dma_start(out=out[:, :], in_=acc[:, :])

    nc.compile()
    _CACHE["nc"] = nc
    return nc


def _make_in_maps(A, X):
    idx = np.arange(NH)
    eye16 = np.eye(128, dtype=np.float16)
    X16 = X[:, 1:4].astype(np.float16)  # (batch, 3, N, N, F)
    in_maps = []
    for c in range(8):
        b, h = c // 2, c % 2
        lo = h * NH
        # [k, i, j, f] -> [k, i, f, j] so j is innermost on the device
        xkT = np.ascontiguousarray(
            X16[b, :, lo : lo + NH].transpose(0, 1, 3, 2)
        )
        av = A[b, lo : lo + NH, :].astype(np.float16)
        dv = np.ascontiguousarray(X[b, 0, lo + idx, lo + idx, :])
        in_maps.append({"xk": xkT, "a": av, "d": dv, "eye": eye16})
    return in_maps


def run(A, X, trace=False, **kw):
    nc = _build_nc()
    in_maps = _make_in_maps(A, X)
    res = run_bass_kernel_spmd(
        nc, in_maps, core_ids=list(range(8)), trace=trace, **kw
    )
    out = np.empty((BATCH, N, F), dtype=np.float32)
    for c in range(8):
        b, h = c // 2, c % 2
        out[b, h * NH : (h + 1) * NH] = res.results[c]["out"]
    return out, res


def kernel(A, X):
    A = np.asarray(A, dtype=np.float32)
    X = np.asarray(X, dtype=np.float32)
    out, _ = run(A, X, trace=False)
    return out


# revision 15
# speedup vs baseline: 1.1286x; 1.0110x over previous
"""GNN message-passing kernel for Trainium2 (8 NeuronCores).

Reference computation:
    out[b,i,f] = X[b,0,i,i,f] + sum_{k=1..3} sum_j A[b,i,j] * X[b,k,i,j,f]

Sharding: 8 cores = (batch b in 0..3) x (i-half h in 0..1); each core owns
a (b, 128-row i-slab) of the output. Hop 0 only contributes its diagonal,
so only X[b,1:4] plus the hop-0 diagonal rows are sent to the device.

Precision: the harness gate is rel_err < 2e-2; fp32 is ~1.9e-7, so X and A
are downcast to fp16 on the host (~6e-4 final rel err). This halves HBM
traffic (12.6 MB/core) and unlocks the DVE's 2x packed mode.

Layout: X slabs are pre-transposed on the host to f-major [k, i, f, j] so
the j-reduction is innermost-contiguous.

DMA: a single deeply-backlogged SP hardware queue sustains ~410 GB/s
(16 shared engines x ~26 GB/s burst). Each f-chunk's three hop slabs are
fused into ONE dma_start (3D access pattern over k) so the ~9-entry
in-flight DMA window covers the whole stream; all dispatches are issued
up front in chunk order.

Compute per f-chunk:
  - PE sums the 3 hops via identity-stationary fp16 matmuls (single-pass,
    full rate) accumulating into PSUM fp32, 512-col slices.
  - ACT copies PSUM -> SBUF fp16 per 2048-col piece.
  - DVE multiplies by A broadcast over f (2x mode: broadcast rides the
    y-dim) in place, then reduces over j with a 3-level contiguous add-tree
    (2x mode; TENSOR_REDUCE has no packed mode so it only handles the final
    32->1 step) and adds the hop-0 diagonal into the fp32 accumulator.
"""

import sys

if "/opt/trn_rl_repo" not in sys.path:
    sys.path.insert(0, "/opt/trn_rl_repo")

import numpy as np

import concourse.bacc as bacc
import concourse.bass as bass
import concourse.mybir as mybir
from concourse.bass_utils import run_bass_kernel_spmd
from concourse.tile import TileContext

BATCH, KP1, N, F = 4, 4, 256, 64
NH = N // 2          # 128 rows of output per core (partition dim)
# f-chunk sizes (sum = F). Small first chunks -> fast pipeline fill +
# HAM ramp; small last chunks -> short drain tail.
CFS = [2, 4, 8, 8, 8, 8, 8, 8, 6, 2, 2]
PIECE = 2048         # PSUM tile free width (8 KB fp32 = 4 banks)
FP32 = mybir.dt.float32
FP16 = mybir.dt.float16

_CACHE = {}


def _build_nc():
    if "nc" in _CACHE:
        return _CACHE["nc"]
    nc = bacc.Bacc("TRN2", target_bir_lowering=False, debug=False, num_devices=8)
    xk = nc.dram_tensor("xk", [3, NH, F, N], FP16, kind="ExternalInput").ap()
    a = nc.dram_tensor("a", [NH, N], FP16, kind="ExternalInput").ap()
    d = nc.dram_tensor("d", [NH, F], FP32, kind="ExternalInput").ap()
    eye = nc.dram_tensor("eye", [128, 128], FP16, kind="ExternalInput").ap()
    out = nc.dram_tensor("out", [NH, F], FP32, kind="ExternalOutput").ap()

    with TileContext(nc) as tc:
        with (
            tc.tile_pool(name="const", bufs=1) as cpool,
            tc.tile_pool(name="xs", bufs=1) as xpool,
            tc.tile_pool(name="sm", bufs=4) as spool,
            tc.tile_pool(name="tr", bufs=3) as tpool,
            tc.tile_pool(name="ac", bufs=1) as acpool,
            tc.tile_pool(name="ps", bufs=2, space="PSUM") as pspool,
        ):
            eye_sb = cpool.tile([128, 128], FP16)
            nc.sync.dma_start(out=eye_sb[:, :], in_=eye[:, :])
            a_sb = cpool.tile([128, N], FP16)
            d_sb = cpool.tile([128, F], FP32)

            acc = acpool.tile([128, F], FP32)
            a_step = a_sb.ap[0][0]

            # One fused DMA per chunk: 3D AP pulls all three hop slabs.
            # All X dispatches go out up front on the SP queue in chunk
            # order. a/d are interleaved after the first X chunks: they
            # are not needed until the first DVE mul / accumulate.
            xts = []
            f0 = 0
            for c, CF in enumerate(CFS):
                CW = CF * N
                xt = xpool.tile([128, 3 * CW], FP16, name=f"x{c}", tag=f"x{c}")
                xt_step = xt.ap[0][0]
                dst = bass.AP(xt.tensor, 0, [[xt_step, 128], [CW, 3], [1, CW]])
                src = bass.AP(
                    xk.tensor,
                    f0 * N,
                    [[F * N, 128], [NH * F * N, 3], [1, CW]],
                )
                nc.sync.dma_start(out=dst, in_=src)
                xts.append(xt)
                f0 += CF
                if c == 1:
                    nc.sync.dma_start(out=a_sb[:, :], in_=a[:, :])
                    nc.sync.dma_start(out=d_sb[:, :], in_=d[:, :])

            # PE warm-up while the first X DMAs are in flight: trips the
            # HAM activity window and bridges the fill gap so real matmuls
            # run at full clock from the start.
            warm = pspool.tile([128, 512], FP32, name="ps", tag="ps")
            for _ in range(26):
                nc.tensor.matmul(
                    warm[:, 0:128], eye_sb[:, :], eye_sb[:, :],
                    start=True, stop=True,
                )

            f0 = 0
            for c, CF in enumerate(CFS):
                CW = CF * N
                xt = xts[c]
                # hop sum on PE -> PSUM fp32, then ACT -> one fp16 tile
                sm = spool.tile([128, CW], FP16, name="sm", tag="sm")
                sm_step = sm.ap[0][0]
                for p0 in range(0, CW, PIECE):
                    pw = min(PIECE, CW - p0)
                    ps = pspool.tile([128, pw], FP32, name="ps", tag="ps")
                    for s in range(pw // 512):
                        sl = slice(s * 512, (s + 1) * 512)
                        for k in range(3):
                            o = k * CW + p0 + s * 512
                            nc.tensor.matmul(
                                ps[:, sl],
                                eye_sb[:, :],
                                xt[:, o : o + 512],
                                start=(k == 0),
                                stop=(k == 2),
                            )
                    nc.scalar.copy(sm[:, p0 : p0 + pw], ps[:, :])

                # sm[i, f*N + j] *= A[i, j]  (in place, 2x mode)
                smi = bass.AP(sm.tensor, 0, [[sm_step, 128], [N, CF], [1, N]])
                ab = bass.AP(a_sb.tensor, 0, [[a_step, 128], [0, CF], [1, N]])
                nc.vector.tensor_mul(smi, smi, ab)

                # j-reduction: 3 tree levels (2x mode) + TENSOR_REDUCE 32->1
                tree = tpool.tile([128, 224 * CF], FP16, name="tree", tag="tree")
                t_step = tree.ap[0][0]
                src_t, src_step, src_off, run = sm.tensor, sm_step, 0, N
                dst_off = 0
                for _ in range(3):
                    half = run // 2
                    i0 = bass.AP(
                        src_t, src_off, [[src_step, 128], [run, CF], [1, half]]
                    )
                    i1 = bass.AP(
                        src_t, src_off + half,
                        [[src_step, 128], [run, CF], [1, half]],
                    )
                    o = bass.AP(
                        tree.tensor, dst_off,
                        [[t_step, 128], [half, CF], [1, half]],
                    )
                    nc.vector.tensor_add(o, i0, i1)
                    src_t, src_step, src_off = tree.tensor, t_step, dst_off
                    dst_off += half * CF
                    run = half

                rin = bass.AP(
                    src_t, src_off, [[src_step, 128], [run, CF], [1, run]]
                )
                nc.vector.reduce_sum(
                    acc[:, f0 : f0 + CF], rin, axis=mybir.AxisListType.X
                )
                f0 += CF

            # single hop-0 diagonal add, then write out
            nc.vector.tensor_add(acc[:, :], acc[:, :], d_sb[:, :])
            nc.sync.dma_start(out=out[:, :], in_=acc[:, :])

    nc.compile()
    _CACHE["nc"] = nc
    return nc


def _make_in_maps(A, X):
    idx = np.arange(NH)
    eye16 = np.eye(128, dtype=np.float16)
    X16 = X[:, 1:4].astype(np.float16)  # (batch, 3, N, N, F)
    in_maps = []
    for c in range(8):
        b, h = c // 2, c % 2
        lo = h * NH
        # [k, i, j, f] -> [k, i, f, j] so j is innermost on the device
        xkT = np.ascontiguousarray(
            X16[b, :, lo : lo + NH].transpose(0, 1, 3, 2)
        )
        av = A[b, lo : lo + NH, :].astype(np.float16)
        dv = np.ascontiguousarray(X[b, 0, lo + idx, lo + idx, :])
        in_maps.append({"xk": xkT, "a": av, "d": dv, "eye": eye16})
    return in_maps


def run(A, X, trace=False, **kw):
    nc = _build_nc()
    in_maps = _make_in_maps(A, X)
    res = run_bass_kernel_spmd(
        nc, in_maps, core_ids=list(range(8)), trace=trace, **kw
    )
    out = np.empty((BATCH, N, F), dtype=np.float32)
    for c in range(8):
        b, h = c // 2, c % 2
        out[b, h * NH : (h + 1) * NH] = res.results[c]["out"]
    return out, res


def kernel(A, X):
    A = np.asarray(A, dtype=np.float32)
    X = np.asarray(X, dtype=np.float32)
    out, _ = run(A, X, trace=False)
    return out
